# revision 13
# baseline (speedup 1.0000x reference)
"""Trainium2 Bass kernel for 3-layer GAT (nn_MultiLayerGAT), v3.

Strategy (dst-node sharding, 8 cores, fp16 data path):
  - Add self-loops, sort edges by dst. Nodes padded to 10240 = 80 blocks of
    128; core k owns blocks [10k, 10k+10). Each block's edges padded to CK
    chunks of 128 slots (same CK for all cores => one SPMD program).
  - Phase A is SHARDED: each core computes xp_ext = h @ W_ext for its own 10
    blocks only, writing two [640, row] fp16 half-shards; two AllGathers
    (issued as soon as each half is complete, overlapping phase B of the
    previous layer) replicate the full xe to every core.  xe row =
    [xp | al_s | pad] fp16; al_d stays in a per-core resident SBUF tile.
    xe_full row layout is region-major: node (c, b, p) lives at row
    c*640 + b*128 + p for blocks b<5, and 5120 + c*640 + (b-5)*128 + p for
    b>=5 (so each AllGather writes one contiguous region); gather indices
    are host-remapped accordingly.
  - Phase B per dst block:
      The TRANSPOSED dst one-hot (ohdT[q, slot] = dstcode[slot]==q) is
      host-precomputed and DMA-streamed; the aggregation one-hot ohd is
      derived on-chip by PE transposes (batched PSUM->SBUF copies).
      al_d per edge slot = ohdT^T @ al_d_blk on TensorE (no dma_gather).
      ONE gpsimd dma_gather per block fetches [xp|al_s] rows by src (768B
      fp16 elements).  Gathers rotate across 4 SWDGE queues so descriptor
      generation runs concurrently on different Q7 core pairs (desc-gen at
      ~41us/4352 idxs is otherwise the serial critical path; measured 2.5x
      faster with 4 queues).
      ee = exp(lrelu(al_s + al_d)); scale gathered xp by ee; segment-sum
      AND the ee denominator via interleaved one-hot matmuls on TensorE
      (fp16, fp32 PSUM): ps[:,0:fe] += ohd_c^T @ (ee*G)_c and
      ps[:,fe:fe+nh] += ohd_c^T @ ee_c.
      h = ps*recip(s+eps) + bias; layers 1-2 store h' = elu(h)+1 = max(h,0)
      + exp(min(h,0)) (the -1 is folded into the next layer's phase A as a
      host-precomputed column-sum correction row), transpose into the
      resident h^T, and run the next layer's phase A for this block
      immediately.
  - Final layer: single head, fp16 rows of 128 cols, log_softmax per block.

Pads: gather idx 0 (finite), dstcode -1 => one-hot column all zero, so pads
contribute nothing to numerator or denominator.
"""

import os
import numpy as np

N = 10000
E = 320000
IN = 128
HID = 32
HEADS = 8
HC = HEADS * HID          # 256
OUT = 40
NEG = 0.2

NPAD = 10240              # 80 blocks of 128
NBLK_TOT = NPAD // 128    # 80
NCORES = 8
NB = NBLK_TOT // NCORES   # 10 blocks per core
NBH = NB // 2             # blocks per half-shard

ROW12 = 384               # xe row fp16 cols, layers 1-2: [xp 256 | al_s 8 | pad]
ROW3 = 128                # layer 3: [xp 40 | al_s 1 | pad]

NQ = int(os.environ.get("GAT_QUEUES", "4"))      # SWDGE queues for gathers
GBUFS = int(os.environ.get("GAT_GBUFS", "4"))    # gather tiles in flight


def rowmap(node):
    """xe_full row of a global (padded) node id, region-major halves."""
    c, r = np.divmod(node, NB * 128)
    b, p = np.divmod(r, 128)
    lo = b < NBH
    return np.where(lo,
                    c * (NBH * 128) + b * 128 + p,
                    NCORES * NBH * 128 + c * (NBH * 128) + (b - NBH) * 128 + p)


# ----------------------------------------------------------------------------
# host-side preprocessing
# ----------------------------------------------------------------------------

def build_w_ext(W, a_src, a_dst, row):
    """W_ext[in, cols]: [W | W@As | W@Ad] (block-diag attention vectors)."""
    inn, hc = W.shape
    H, C = a_src.shape
    As = np.zeros((hc, H), np.float32)
    Ad = np.zeros((hc, H), np.float32)
    for h in range(H):
        As[h * C:(h + 1) * C, h] = a_src[h]
        Ad[h * C:(h + 1) * C, h] = a_dst[h]
    We = np.zeros((inn, row), np.float32)
    We[:, 0:hc] = W
    We[:, hc:hc + H] = W @ As
    We[:, hc + H:hc + 2 * H] = W @ Ad
    return We.astype(np.float16)


def preprocess(edge_index):
    """Chunk tables shared by all layers. Returns (CK, per-core arrays)."""
    src = np.concatenate([edge_index[0], np.arange(N, dtype=edge_index.dtype)])
    dst = np.concatenate([edge_index[1], np.arange(N, dtype=edge_index.dtype)])
    src = src.astype(np.int64)
    dst = dst.astype(np.int64)
    order = np.argsort(dst, kind="stable")
    ssrc, sdst = src[order], dst[order]
    blk = sdst // 128
    cnt = np.bincount(blk, minlength=NBLK_TOT)
    CK = int(np.ceil(cnt.max() / 128))
    S = CK * 128
    starts = np.concatenate([[0], np.cumsum(cnt)])

    gsrc = np.zeros((NBLK_TOT, S), np.int64)           # gather row (by src)
    dstc = np.full((NBLK_TOT, S), -1, np.int32)        # dst - 128*block
    srows = rowmap(ssrc)
    for b in range(NBLK_TOT):
        lo, hi = starts[b], starts[b + 1]
        n = hi - lo
        gsrc[b, :n] = srows[lo:hi]
        dstc[b, :n] = (sdst[lo:hi] - 128 * b).astype(np.int32)

    def wrap16(idx_flat):
        t16 = idx_flat.reshape(S // 16, 16).T.astype(np.int16)
        return np.tile(t16, (8, 1))

    qvec = np.arange(128, dtype=np.int32).reshape(128, 1)
    cores = []
    for k in range(NCORES):
        bs = range(k * NB, (k + 1) * NB)
        gsrc_t = np.concatenate([wrap16(gsrc[b]) for b in bs], axis=1)
        # ohdT[q, (b*CK + c)*128 + p] = (dstc[b, c*128+p] == q), fp16
        codes = dstc[k * NB:(k + 1) * NB].reshape(1, NB * S)
        ohdT = (qvec == codes).astype(np.float16)
        cores.append(dict(gsrc=gsrc_t, ohdT=np.ascontiguousarray(ohdT)))
    return CK, cores


# ----------------------------------------------------------------------------
# bass program
# ----------------------------------------------------------------------------

def build_nc(CK):
    import concourse.bacc as bacc
    import concourse.mybir as mybir
    import concourse.tile as tile
    from concourse.library_config import mlp

    f32 = mybir.dt.float32
    f16 = mybir.dt.float16
    i16 = mybir.dt.int16
    Alu = mybir.AluOpType
    Act = mybir.ActivationFunctionType

    S = CK * 128
    HR = NBH * 128            # rows per half-shard (640)

    nc = bacc.Bacc("TRN2", debug=False, num_swdge_queues=NQ)

    # inputs (per core)
    xT16 = nc.dram_tensor("xT16", [IN, NB * 128], f16, kind="ExternalInput")
    xTf = nc.dram_tensor("xTf", [IN, NPAD], f16, kind="ExternalInput")
    W1e = nc.dram_tensor("W1e", [IN, 272], f16, kind="ExternalInput")
    W2e = nc.dram_tensor("W2e", [HC, 272], f16, kind="ExternalInput")
    W3e = nc.dram_tensor("W3e", [HC, 64], f16, kind="ExternalInput")
    gsrc = nc.dram_tensor("gsrc", [128, NB * S // 16], i16, kind="ExternalInput")
    ohdT = nc.dram_tensor("ohdT", [128, NB * S], f16, kind="ExternalInput")
    ident = nc.dram_tensor("ident", [128, 128], f16, kind="ExternalInput")
    b1r = nc.dram_tensor("b1r", [128, HC], f32, kind="ExternalInput")
    b2r = nc.dram_tensor("b2r", [128, HC], f32, kind="ExternalInput")
    b3r = nc.dram_tensor("b3r", [128, OUT], f32, kind="ExternalInput")
    c2r = nc.dram_tensor("c2r", [128, 272], f32, kind="ExternalInput")
    c3r = nc.dram_tensor("c3r", [128, 64], f32, kind="ExternalInput")

    out = nc.dram_tensor("out", [NB * 128, OUT], f32, kind="ExternalOutput")

    # scratch DRAM: two half-shards + two-region full tensors per layer
    def xe_pair(name, row):
        sa = nc.dram_tensor(name + "sa", [HR, row], f16)
        sb = nc.dram_tensor(name + "sb", [HR, row], f16)
        ff = nc.dram_tensor(name + "f", [NPAD, row], f16, addr_space="Shared")
        return sa, sb, ff

    xe1f = nc.dram_tensor("xe1f", [NPAD, ROW12], f16)
    xe2sa, xe2sb, xe2f = xe_pair("xe2", ROW12)
    xe3sa, xe3sb, xe3f = xe_pair("xe3", ROW3)

    with tile.TileContext(nc) as tc:
        nc.gpsimd.load_library(mlp)
        with tc.tile_pool(name="a1", bufs=4) as a1pool, \
             tc.tile_pool(name="const", bufs=1) as cpool, \
             tc.tile_pool(name="res", bufs=1) as rpool, \
             tc.tile_pool(name="oht", bufs=3) as ohtpool, \
             tc.tile_pool(name="ohd", bufs=2) as ohpool, \
             tc.tile_pool(name="g", bufs=GBUFS) as gpool, \
             tc.tile_pool(name="g3", bufs=3) as g3pool, \
             tc.tile_pool(name="small", bufs=2) as spool, \
             tc.tile_pool(name="post", bufs=2) as ppool, \
             tc.tile_pool(name="psA", bufs=2, space="PSUM") as psA, \
             tc.tile_pool(name="psB", bufs=2, space="PSUM") as psB, \
             tc.tile_pool(name="psT", bufs=2, space="PSUM") as psT, \
             tc.tile_pool(name="psAD", bufs=2, space="PSUM") as psAD:

            gsrc_t = cpool.tile([128, NB * S // 16], i16, tag="gsrc")
            nc.sync.dma_start(gsrc_t[:], gsrc[:])
            ident_t = cpool.tile([128, 128], f16, tag="ident")
            nc.sync.dma_start(ident_t[:], ident[:])
            xT_t = cpool.tile([128, NB * 128], f16, tag="xT")
            nc.sync.dma_start(xT_t[:], xT16[:])

            w1_t = cpool.tile([128, 272], f16, tag="w1")
            nc.sync.dma_start(w1_t[:], W1e[:])
            w2_t = cpool.tile([128, 2, 272], f16, tag="w2")
            for kk in range(2):
                nc.sync.dma_start(w2_t[:, kk, :], W2e[kk * 128:(kk + 1) * 128, :])
            w3_t = cpool.tile([128, 2, 64], f16, tag="w3")
            for kk in range(2):
                nc.sync.dma_start(w3_t[:, kk, :], W3e[kk * 128:(kk + 1) * 128, :])
            b1_t = cpool.tile([128, HC], f32, tag="b1")
            nc.sync.dma_start(b1_t[:], b1r[:])
            b2_t = cpool.tile([128, HC], f32, tag="b2")
            nc.sync.dma_start(b2_t[:], b2r[:])
            b3_t = cpool.tile([128, OUT], f32, tag="b3")
            nc.sync.dma_start(b3_t[:], b3r[:])
            c2_t = cpool.tile([128, 272], f32, tag="c2")
            nc.sync.dma_start(c2_t[:], c2r[:])
            c3_t = cpool.tile([128, 64], f32, tag="c3")
            nc.sync.dma_start(c3_t[:], c3r[:])

            hT_t = rpool.tile([128, 2, NB, 128], f16, tag="hT")
            ald1_t = rpool.tile([128, NB, HEADS], f16, tag="ald1")
            ald2_t = rpool.tile([128, NB, HEADS], f16, tag="ald2")
            ald3_t = rpool.tile([128, NB, 1], f16, tag="ald3")

            def shard_write(xe_sa, xe_sb, b, src_ap, wr):
                tgt = xe_sa if b < NBH else xe_sb
                r0 = (b % NBH) * 128
                nc.sync.dma_start(tgt[r0:r0 + 128, 0:wr], src_ap)

            def allgather(src_dram, dst_ap):
                nc.gpsimd.collective_compute(
                    "AllGather", mybir.AluOpType.bypass,
                    replica_groups=[list(range(NCORES))],
                    ins=[src_dram.ap().opt()], outs=[dst_ap.opt()])

            def phase_a1():
                # replicated: every core computes the full xe1 locally
                for t in range(NBLK_TOT):
                    c, b = divmod(t, NB)
                    row0 = (c * HR + b * 128 if b < NBH
                            else NCORES * HR + c * HR + (b - NBH) * 128)
                    lhs = a1pool.tile([128, 128], f16, tag="lhsA")
                    nc.sync.dma_start(lhs[:], xTf[:, t * 128:(t + 1) * 128])
                    ps = psA.tile([128, 272], f32, tag="psA")
                    nc.tensor.matmul(ps[:], lhs[:],
                                     w1_t[:], start=True, stop=True)
                    xa = a1pool.tile([128, 264], f16, tag="xeA1")
                    if t % 2 == 0:
                        nc.scalar.activation(xa[:], ps[:, 0:264], Act.Copy)
                    else:
                        nc.vector.tensor_copy(xa[:], ps[:, 0:264])
                    nc.sync.dma_start(xe1f[row0:row0 + 128, 0:264], xa[:])
                # own blocks only: al_d1 columns
                for t in range(NB):
                    ps = psAD.tile([128, CK, HEADS], f32, tag="adps")
                    nc.tensor.matmul(ps[:, 0, :], xT_t[:, t * 128:(t + 1) * 128],
                                     w1_t[:, 264:272], start=True, stop=True)
                    nc.vector.tensor_copy(ald1_t[:, t, :], ps[:, 0, :])

            def phase_b(L, xe_f, ald_t, b_t, wn_t, corr_t, ald_next,
                        xe_nsa, xe_nsb):
                nh = HEADS if L < 3 else 1
                fe = HC if L < 3 else OUT
                row = ROW12 if L < 3 else ROW3
                for b in range(NB):
                    ofs = b * S
                    # transposed one-hot from host; derive ohd by PE transpose
                    oht = ohtpool.tile([128, CK, 128], f16, tag="oht")
                    nc.sync.dma_start(
                        oht[:].rearrange("p c q -> p (c q)"),
                        ohdT[:, ofs:ofs + S])
                    ohd = ohpool.tile([128, CK, 128], f16, tag="ohd")
                    for grp in range((CK + 3) // 4):
                        c0 = grp * 4
                        cw = min(4, CK - c0)
                        pt = psT.tile([128, 4, 128], f16, tag="tr4")
                        for j in range(cw):
                            nc.tensor.transpose(
                                pt[:, j, :], oht[:, c0 + j, :], ident_t[:])
                        nc.scalar.activation(
                            ohd[:, c0:c0 + cw, :], pt[:, 0:cw, :], Act.Copy)
                    # al_d per edge slot = ohdT^T @ al_d_blk  (TensorE)
                    adps = psAD.tile([128, CK, HEADS], f32, tag="adps")
                    for c in range(CK):
                        nc.tensor.matmul(adps[:, c, 0:nh], oht[:, c, :],
                                         ald_t[:, b, 0:nh],
                                         start=True, stop=True)
                    # THE gather: [xp | al_s] rows by src
                    g = (gpool if L < 3 else g3pool).tile(
                        [128, CK, row], f16, tag="g" if L < 3 else "g3")
                    NP = min(4, NQ)
                    cuts = [round(p * CK / NP) for p in range(NP + 1)]
                    for p in range(NP):
                        c0, c1 = cuts[p], cuts[p + 1]
                        sh = (c1 - c0) * 128
                        isl = gsrc_t[:, (b * S + c0 * 128) // 16:
                                     (b * S + c1 * 128) // 16]
                        nc.gpsimd.dma_gather(
                            g[:, c0:c1, :], xe_f[:, 0:row],
                            isl, sh, sh, row, elem_step=row,
                            single_packet=False,
                            queue_num=(NP * b + p) % NQ)
                    # ee = exp(lrelu(al_s + al_d))
                    z = spool.tile([128, CK, nh], f32, tag="z")
                    nc.vector.tensor_tensor(
                        z[:], g[:, :, fe:fe + nh], adps[:, :, 0:nh], Alu.add)
                    zf = z[:].rearrange("p c h -> p (c h)")
                    nc.vector.scalar_tensor_tensor(
                        zf, zf, NEG, zf, Alu.mult, Alu.max)
                    ee = spool.tile([128, CK, nh], f16, tag="ee")
                    nc.scalar.activation(
                        ee[:].rearrange("p c h -> p (c h)"), zf, Act.Exp)
                    # scale features by ee
                    if L < 3:
                        nc.vector.tensor_tensor(
                            g[:, :, 0:fe].rearrange("p c (h w) -> p c h w", w=HID),
                            g[:, :, 0:fe].rearrange("p c (h w) -> p c h w", w=HID),
                            ee[:].to_broadcast([128, CK, nh, HID]),
                            Alu.mult)
                    else:
                        nc.vector.tensor_tensor(
                            g[:, :, 0:fe], g[:, :, 0:fe],
                            ee[:].rearrange("p c h -> p (c h)").to_broadcast(
                                [128, CK, fe]),
                            Alu.mult)
                    # segment-sum + ee denominator via interleaved matmuls
                    ps = psB.tile([128, 264], f32, tag="agg")
                    for c in range(CK):
                        nc.tensor.matmul(
                            ps[:, 0:fe], ohd[:, c, :], g[:, c, 0:fe],
                            start=(c == 0), stop=(c == CK - 1))
                    for c in range(CK):
                        nc.tensor.matmul(
                            ps[:, fe:fe + nh], ohd[:, c, :], ee[:, c, :],
                            start=(c == 0), stop=(c == CK - 1))
                    # h = ps * 1/(s+eps) + bias
                    r = spool.tile([128, nh], f32, tag="r")
                    nc.vector.reciprocal(r[:], ps[:, fe:fe + nh])
                    h = ppool.tile([128, fe], f32, tag="h")
                    if L < 3:
                        nc.vector.tensor_tensor(
                            h[:].rearrange("p (x w) -> p x w", w=HID),
                            ps[:, 0:fe].rearrange("p (x w) -> p x w", w=HID),
                            r[:].to_broadcast([128, nh, HID]),
                            Alu.mult)
                        nc.vector.tensor_tensor(h[:], h[:], b_t[:], Alu.add)
                        # h' = elu(h)+1 = max(h,0) + exp(min(h,0))
                        t2 = ppool.tile([128, fe], f32, tag="elu")
                        nc.vector.scalar_tensor_tensor(
                            t2[:], h[:], 0.0, h[:], Alu.min, Alu.min)
                        nc.scalar.activation(t2[:], t2[:], Act.Exp)
                        h16 = ppool.tile([128, fe], f16, tag="h16")
                        nc.vector.scalar_tensor_tensor(
                            h16[:], h[:], 0.0, t2[:], Alu.max, Alu.add)
                        for half in range(2):
                            pt = psT.tile([128, 4, 128], f16, tag="tr4")
                            nc.tensor.transpose(
                                pt[:, 0, :], h16[:, half * 128:(half + 1) * 128],
                                ident_t[:])
                            nc.vector.tensor_copy(hT_t[:, half, b, :],
                                                  pt[:, 0, :])
                        # embedded next-layer phase A (h' @ W - colsum(W))
                        ncols = 272 if L == 1 else 64
                        wr = 264 if L == 1 else 41
                        nhn = 8 if L == 1 else 1
                        psa = psA.tile([128, 272], f32, tag="psA")
                        for kk in range(2):
                            nc.tensor.matmul(
                                psa[:, 0:ncols], hT_t[:, kk, b, :],
                                wn_t[:, kk, 0:ncols],
                                start=(kk == 0), stop=(kk == 1))
                        xa = ppool.tile([128, 264], f16, tag="xeA")
                        nc.vector.tensor_tensor(
                            xa[:, 0:wr], psa[:, 0:wr], corr_t[:, 0:wr],
                            Alu.subtract)
                        nc.vector.tensor_tensor(
                            ald_next[:, b, 0:nhn], psa[:, wr:wr + nhn],
                            corr_t[:, wr:wr + nhn], Alu.subtract)
                        shard_write(xe_nsa, xe_nsb, b, xa[:, 0:wr], wr)
                        if b == NBH - 1:
                            allgather(xe_nsa,
                                      (xe2f if L == 1 else xe3f)[0:NCORES * HR, :])
                        elif b == NB - 1:
                            allgather(xe_nsb,
                                      (xe2f if L == 1 else xe3f)[NCORES * HR:NPAD, :])
                    else:
                        # log_softmax over the 40 classes
                        nc.vector.tensor_tensor(
                            h[:], ps[:, 0:fe],
                            r[:].to_broadcast([128, fe]), Alu.mult)
                        nc.vector.tensor_tensor(h[:], h[:], b_t[:], Alu.add)
                        m_t = spool.tile([128, 1], f32, tag="m")
                        nc.vector.tensor_reduce(
                            m_t[:], h[:], mybir.AxisListType.X, Alu.max)
                        nc.vector.tensor_tensor(
                            h[:], h[:], m_t[:].to_broadcast([128, fe]),
                            Alu.subtract)
                        x_t = ppool.tile([128, fe], f32, tag="exps")
                        s_t = spool.tile([128, 1], f32, tag="s")
                        nc.scalar.activation(
                            x_t[:], h[:], Act.Exp, accum_out=s_t[:])
                        l_t = spool.tile([128, 1], f32, tag="l")
                        nc.scalar.activation(l_t[:], s_t[:], Act.Ln)
                        nc.vector.tensor_tensor(
                            h[:], h[:], l_t[:].to_broadcast([128, fe]),
                            Alu.subtract)
                        nc.sync.dma_start(out[b * 128:(b + 1) * 128, :], h[:])

            phase_a1()
            phase_b(1, xe1f, ald1_t, b1_t, w2_t, c2_t, ald2_t, xe2sa, xe2sb)
            phase_b(2, xe2f, ald2_t, b2_t, w3_t, c3_t, ald3_t, xe3sa, xe3sb)
            phase_b(3, xe3f, ald3_t, b3_t, None, None, None, None, None)

    nc.compile()
    return nc


# ----------------------------------------------------------------------------
# entry point
# ----------------------------------------------------------------------------

LAST_EXEC_NS = None


def kernel(**inputs):
    from concourse.bass_utils import run_bass_kernel_spmd
    global LAST_EXEC_NS

    x = np.asarray(inputs["x"], np.float32)
    ei = np.asarray(inputs["edge_index"])
    CK, cores = preprocess(ei)

    xT16 = np.zeros((IN, NPAD), np.float16)
    xT16[:, 0:N] = x.T.astype(np.float16)
    W1en = build_w_ext(np.asarray(inputs["W1"], np.float32),
                       np.asarray(inputs["a_src1"], np.float32),
                       np.asarray(inputs["a_dst1"], np.float32), 272)
    W2en = build_w_ext(np.asarray(inputs["W2"], np.float32),
                       np.asarray(inputs["a_src2"], np.float32),
                       np.asarray(inputs["a_dst2"], np.float32), 272)
    W3_ = np.asarray(inputs["W3"], np.float32)
    W3en = np.zeros((HC, 64), np.float32)
    W3en[:, 0:OUT] = W3_
    W3en[:, OUT:OUT + 1] = W3_ @ np.asarray(inputs["a_src3"], np.float32).reshape(OUT, 1)
    W3en[:, OUT + 1:OUT + 2] = W3_ @ np.asarray(inputs["a_dst3"], np.float32).reshape(OUT, 1)
    W3en = W3en.astype(np.float16)

    # ELU -1 fold: colsum correction rows for the next layer's W_ext
    c2n = np.tile(W2en.astype(np.float32).sum(axis=0), (128, 1)).astype(np.float32)
    c3n = np.tile(W3en.astype(np.float32).sum(axis=0), (128, 1)).astype(np.float32)

    ident_n = np.eye(128, dtype=np.float16)
    b1n = np.tile(np.asarray(inputs["b1"], np.float32), (128, 1))
    b2n = np.tile(np.asarray(inputs["b2"], np.float32), (128, 1))
    b3n = np.tile(np.asarray(inputs["b3"], np.float32), (128, 1))

    nc = build_nc(CK)
    in_maps = []
    for k in range(NCORES):
        in_maps.append({
            "xT16": np.ascontiguousarray(xT16[:, k * NB * 128:(k + 1) * NB * 128]),
            "xTf": xT16,
            "W1e": W1en, "W2e": W2en, "W3e": W3en,
            "gsrc": cores[k]["gsrc"], "ohdT": cores[k]["ohdT"],
            "ident": ident_n,
            "b1r": b1n, "b2r": b2n, "b3r": b3n, "c2r": c2n, "c3r": c3n,
        })
    trace = bool(int(os.environ.get("GAT_TRACE", "0")))
    res = run_bass_kernel_spmd(nc, in_maps, list(range(NCORES)), trace=trace)
    LAST_EXEC_NS = res.exec_time_ns
    full = np.concatenate([res.results[k]["out"] for k in range(NCORES)], axis=0)
    return full[0:N].astype(np.float32)
